# revision 5
# baseline (speedup 1.0000x reference)
"""Trainium2 Bass kernel for nn_AdExternal_N3Tree (gnn_message_passing).

Key insight: the reference's sequential 32768-step scan only affects the
output through `acc` (a 32-vector). Each parent's 8-child sibling group is an
independent serial chain that is LINEAR in that parent's original 8x32 block,
and group depth is constant within each of 6 contiguous parent-index classes.
So  acc = sum_d M_d @ s_d + gamma_tot,  where s_d is the sum of flattened
parent blocks over class d (a pure 4MB reduction) and M_d (32x256) / gamma
are tiny matrices computed on the host from conv_w/conv_b/depth_weight alone.

The leaf cells that feed the MLPs (flat cells 32767..262143) are never
written by the scan, so  out[leaf] = MLP(data_flat[leaf] + acc)  and cells
0..32766 are zero.

Device work per core (SPMD on 8 cores, no collectives - each core is fully
independent, which also makes the kernel immune to cross-core launch skew):
  - scan DMA split into 8 slices over the 3 DMA-capable queues (sync HWDGE,
    scalar HWDGE, gpsimd SWDGE) so the class-sum stage starts as slices land
  - stage 1: class sums with two PE quadrants: mixed-class node tiles cycle
    indicator weights in quadrant 0; the 26 pure-class-4 tiles reuse ONE
    resident indicator in quadrant 1 (LDWEIGHTS elision -> feed-bound)
  - tiny chain in bf16: s -> transpose (R-matrix matmuls fold both quadrant
    row groups) -> even/odd acc quadrants -> widened [65,128] bias matmul
    merges them for free -> folded layer-1 bias (128,)
  - MLP over a 29184-cell slice in bf16: x@W1cat (row-tiled) -> GELU+bias on
    ScalarE (the throughput bound, f32 PSUM in) -> @W2cat (col-tiled) ->
    +b2 evict on DVE
  - output written as 12 (c,o)-planes; host interleaves/assembles.
"""
import sys

for _p in ("/opt/trn_rl_repo", "/opt/trn_rl_repo/concourse"):
    if _p not in sys.path:
        sys.path.insert(0, _p)

import numpy as np

N_NODES = 32768
D = 32
N_GROUPS = 4096
N_CELLS = N_NODES * 8           # 262144
LEAF0 = N_NODES - 1             # 32767 first leaf cell
CORES = 8
CELLS_MAIN = 28672              # cells each core contributes (core 7: +1)
CELLS_CORE = 29184              # cells each core processes = 19 * 1536
CH = 1536                       # cells per chunk (3 row-tiled sub-chunks of 512)
NCH = 19
SUB = 512
NSUB = 3
SCAN_TILES = 32                 # replicated scan: 32 node-tiles of 128
SCAN_FREE = SCAN_TILES * 256    # 8192
XT_FREE = NCH * SUB             # 9728

# stage-1 tile classification: tiles fully inside class 4 share one
# indicator (nodes 640..3967 = tiles 5..30); the rest are "mixed"
PURE_LO, PURE_HI = 5, 30
MIXED_TILES = [0, 1, 2, 3, 4, 31]

# constsB (bf16, [128, NB]) column map
IND0 = 0                        # ind: 32 tiles x 6 classes
R0 = IND0 + 6 * SCAN_TILES      # 192: R fold matrix [38, 6]
M20 = R0 + 6                    # 198: m2 [128, 12*32]
WB0 = M20 + 384                 # 582: wb2 [65, 128]
W10 = WB0 + 128                 # 710: w1cat3 [96, 128]
W20 = W10 + 128                 # 838: w2 [128, 4]
NB = W20 + 4                    # 842

# (p_lo, p_hi_inclusive, conv_depth, n_children, extra_j0_step)
CLASSES = [
    (0, 0, 1, 8, True),
    (1, 8, 2, 8, False),
    (9, 72, 3, 8, False),
    (73, 584, 4, 8, False),
    (585, 4094, 5, 8, False),
    (4095, 4095, 5, 7, False),
]


# ---------------------------------------------------------------- host math
def _chain(conv_w_d, conv_b_d, dw_d, n_children):
    W = conv_w_d.astype(np.float64)
    b = conv_b_d.astype(np.float64)
    Wk = [W[:, :, k] for k in range(8)]
    A, beta = {}, {}
    if n_children == 7:
        A7 = np.zeros((8, D, D))
        A7[7] = np.eye(D)
        A[7] = A7
        beta[7] = np.zeros(D)
        cs = range(6, -1, -1)
    else:
        cs = range(7, -1, -1)
    for c in cs:
        Ac = np.zeros((8, D, D))
        bc = b.copy()
        for k in range(0, c + 1):
            Ac[k] += Wk[k]
        for m in range(c + 1, 8):
            for k in range(8):
                Ac[k] += Wk[m] @ A[m][k]
            bc += Wk[m] @ beta[m]
        A[c] = Ac
        beta[c] = bc
    Msum = np.zeros((8, D, D))
    gamma = np.zeros(D)
    for c in (range(8) if n_children == 8 else range(7)):
        Msum += dw_d * A[c]
        gamma += dw_d * beta[c]
    return A, beta, Msum, gamma


def _build_class_mats(conv_w, conv_b, depth_weight):
    out = []
    for (p_lo, p_hi, dep, nch, extra) in CLASSES:
        A, beta, Msum, gamma = _chain(
            conv_w[dep], conv_b[dep], float(depth_weight[dep]), nch
        )
        if extra:
            W0 = conv_w[0].astype(np.float64)
            b0 = conv_b[0].astype(np.float64)
            W0k = [W0[:, :, k] for k in range(8)]
            Ae = np.zeros((8, D, D))
            be = b0.copy()
            for m in range(8):
                for k in range(8):
                    Ae[k] += W0k[m] @ A[m][k]
                be += W0k[m] @ beta[m]
            Msum = Msum + float(depth_weight[0]) * Ae
            gamma = gamma + float(depth_weight[0]) * be
        M = np.concatenate([Msum[k] for k in range(8)], axis=1)  # (D, 8D)
        out.append((p_lo, p_hi, M, gamma))
    return out


# ---------------------------------------------------------------- device graph
_GRAPH = None


def _build_graph():
    import concourse.bacc as bacc
    import concourse.mybir as mybir
    from concourse import tile

    F32 = mybir.dt.float32
    BF16 = mybir.dt.bfloat16
    nc = bacc.Bacc("TRN2", target_bir_lowering=False, debug=False, num_devices=CORES)

    cb_d = nc.declare_dram_parameter("cb", [128, NB], BF16, isOutput=False)
    scan_d = nc.declare_dram_parameter("scanX", [128, SCAN_FREE], BF16, isOutput=False)
    xT_d = nc.declare_dram_parameter("xT", [96, XT_FREE], BF16, isOutput=False)
    b2_d = nc.declare_dram_parameter("b2col", [128, 1], F32, isOutput=False)
    out_d = nc.declare_dram_parameter("out", [12, XT_FREE], F32, isOutput=True)

    Gelu = mybir.ActivationFunctionType.Gelu

    with tile.TileContext(nc) as tc:
        with (
            tc.tile_pool(name="const", bufs=1) as cpool,
            tc.tile_pool(name="data", bufs=1) as dpool,
            tc.tile_pool(name="gp", bufs=3) as gpool,
        ):
            # ACT warm-up (gelu table load) + PE warm-up source, no DMA deps
            warm_sb = cpool.tile([1, 8], F32)
            nc.gpsimd.memset(warm_sb[:], 0.0)
            nc.scalar.activation(warm_sb[:], warm_sb[:], Gelu)
            warmd_sb = cpool.tile([32, 128], BF16)
            nc.gpsimd.memset(warmd_sb[:], 0.001)

            cb_sb = cpool.tile([128, NB], BF16)
            b2_sb = cpool.tile([128, 1], F32)
            acc1 = cpool.tile([65, 1], BF16)
            nc.gpsimd.memset(acc1[64:65, :], 1.0)
            bias_sb = cpool.tile([128, 1], F32)
            s_sb = cpool.tile([38, 256], BF16)
            sT_sb = cpool.tile([128, 12], BF16)

            scan_sb = dpool.tile([128, SCAN_FREE], BF16)
            xT_sb = dpool.tile([96, XT_FREE], BF16)
            stage_sb = dpool.tile([128, XT_FREE], F32)

            # ---- DMA enqueues ----
            # consts first on scalar (needed by stage 1); scan slices are
            # interleaved across the 3 queues in consumption order; xT
            # quarters follow their queue's scan work (FIFO per queue keeps
            # the scan at full bandwidth); b2col last (needed at ~first add)
            nc.scalar.dma_start(cb_sb[:], cb_d.ap())
            SLICE_Q = [nc.sync, nc.gpsimd, nc.sync, nc.gpsimd,
                       nc.sync, nc.gpsimd, nc.sync, nc.sync]
            for k, eng in enumerate(SLICE_Q):
                eng.dma_start(
                    scan_sb[:, 1024 * k:1024 * (k + 1)],
                    scan_d.ap()[:, 1024 * k:1024 * (k + 1)],
                )
            XT_Q = [nc.gpsimd, nc.gpsimd, nc.sync, nc.sync]
            for q, eng in enumerate(XT_Q):
                eng.dma_start(
                    xT_sb[:, q * 2432:(q + 1) * 2432],
                    xT_d.ap()[:, q * 2432:(q + 1) * 2432],
                )
            nc.scalar.dma_start(b2_sb[:], b2_d.ap())

            with tc.tile_pool(name="psZ", bufs=2, space="PSUM") as zp:
                with tc.tile_pool(name="psC", bufs=1, space="PSUM") as pchain:
                    # chain PSUM lives in ONE bank: cols 0:256 stage-1 class
                    # sums (+ warm-up junk), 256:268 sT, 268:269 acc E/O,
                    # 269:270 bias
                    ps_part = pchain.tile([128, 272], F32)

                    # PE pre-warm: open the HAM clock gate before stage 1
                    for _ in range(16):
                        nc.tensor.matmul(
                            ps_part[:, 0:128], warmd_sb[:], warmd_sb[:],
                            start=True, stop=True,
                        )

                    # stage 1: class sums over the replicated scan region.
                    # mixed tiles cycle indicators in quadrant 0 (rows 0-5);
                    # pure class-4 tiles share tile-5's indicator resident in
                    # quadrant 1 (rows 32-37) -> no LDWEIGHTS between them
                    for T in range(SCAN_TILES):
                        pure = PURE_LO <= T <= PURE_HI
                        ind_T = 6 * PURE_LO if pure else 6 * T
                        nc.tensor.matmul(
                            ps_part[32:38, 0:256] if pure else ps_part[0:6, 0:256],
                            cb_sb[:, IND0 + ind_T:IND0 + ind_T + 6],
                            scan_sb[:, 256 * T:256 * (T + 1)],
                            start=(T == PURE_LO if pure else T == 0),
                            stop=(T == PURE_HI if pure else T == SCAN_TILES - 1),
                            tile_position=(0, 32) if pure else (0, 0),
                        )

                    # s (38,256) -> sT (128,12) via R-matmuls that also fold
                    # the two quadrant row groups (R[d,d]=R[32+d,d]=1)
                    nc.vector.tensor_copy(s_sb[:], ps_part[0:38, 0:256])
                    for jhi in range(2):
                        nc.tensor.matmul(
                            ps_part[:, 256 + 6 * jhi:256 + 6 * jhi + 6],
                            s_sb[:, 128 * jhi:128 * (jhi + 1)],
                            cb_sb[0:38, R0:R0 + 6],
                            start=True, stop=True,
                        )
                    nc.vector.tensor_copy(sT_sb[:], ps_part[:, 256:268])

                    # acc = sum_k M2_k @ sT[:, k], even k in quadrant 0
                    # (rows 0:32), odd k in quadrant 1 (rows 32:64)
                    for k in range(12):
                        odd = k % 2
                        nc.tensor.matmul(
                            ps_part[32 * odd:32 * odd + 32, 268:269],
                            cb_sb[:, M20 + 32 * k:M20 + 32 * (k + 1)],
                            sT_sb[:, k:k + 1],
                            start=(k < 2), stop=(k >= 10),
                            tile_position=(0, 32 * odd),
                        )
                    nc.vector.tensor_copy(acc1[0:64, :], ps_part[0:64, 268:269])

                    # bias1_eff = W1cat.T@(accE+accO) + (b1cat + gamma@W1cat)
                    # via the widened [65,128] wb2 (rows 0-31 W1, 32-63 W1,
                    # 64 bconst)
                    nc.tensor.matmul(
                        ps_part[:, 269:270], cb_sb[0:65, WB0:WB0 + 128],
                        acc1[:], start=True, stop=True,
                    )
                    nc.vector.tensor_copy(bias_sb[:], ps_part[:, 269:270])

                with tc.tile_pool(name="psO", bufs=2, space="PSUM") as op:
                    for t in range(NCH):
                        z = zp.tile([128, CH], F32)
                        for a in range(NSUB):
                            nc.tensor.matmul(
                                z[:, SUB * a:SUB * (a + 1)],
                                cb_sb[32 * a:32 * (a + 1), W10:W10 + 128],
                                xT_sb[32 * a:32 * (a + 1), SUB * t:SUB * (t + 1)],
                                start=True,
                                stop=True,
                                tile_position=(32 * a, 0),
                            )
                        g = gpool.tile([128, CH], BF16)
                        nc.scalar.activation(g[:], z[:], Gelu, bias=bias_sb[:])
                        o_ps = op.tile([128, SUB], F32)
                        for c in range(NSUB):
                            nc.tensor.matmul(
                                o_ps[32 * c:32 * c + 4, :],
                                cb_sb[:, W20:W20 + 4],
                                g[:, SUB * c:SUB * (c + 1)],
                                start=True,
                                stop=True,
                                tile_position=(0, 32 * c),
                            )
                        nc.vector.tensor_scalar_add(
                            stage_sb[:, SUB * t:SUB * (t + 1)], o_ps[:], b2_sb[:]
                        )
                        # batched output DMA on the idle gpsimd queue
                        if t in (4, 9, 14, 17, NCH - 1):
                            lo = {4: 0, 9: 2560, 14: 5120, 17: 7680,
                                  NCH - 1: 9216}[t]
                            hi = SUB * (t + 1)
                            eng = nc.sync if t == NCH - 1 else nc.gpsimd
                            for c in range(NSUB):
                                eng.dma_start(
                                    out_d.ap()[4 * c:4 * c + 4, lo:hi],
                                    stage_sb[32 * c:32 * c + 4, lo:hi],
                                )

    nc.compile()
    return nc


def _get_graph():
    global _GRAPH
    if _GRAPH is None:
        _GRAPH = _build_graph()
    return _GRAPH


# ---------------------------------------------------------------- kernel
def kernel(**inputs):
    import ml_dtypes
    from concourse import bass_utils

    data = np.asarray(inputs["data"], np.float32)
    conv_w = np.asarray(inputs["conv_w"], np.float32)
    conv_b = np.asarray(inputs["conv_b"], np.float32)
    dw = np.asarray(inputs["depth_weight"], np.float32)
    f_w1 = np.asarray(inputs["f_w1"], np.float32)
    f_b1 = np.asarray(inputs["f_b1"], np.float32)
    f_w2 = np.asarray(inputs["f_w2"], np.float32)
    f_b2 = np.asarray(inputs["f_b2"], np.float32)
    s_w1 = np.asarray(inputs["s_w1"], np.float32)
    s_b1 = np.asarray(inputs["s_b1"], np.float32)
    s_w2 = np.asarray(inputs["s_w2"], np.float32)
    s_b2 = np.asarray(inputs["s_b2"], np.float32)

    # --- weight-derived host constants (no data-sized work here) ---
    mats = _build_class_mats(conv_w, conv_b, dw)

    W1cat = np.concatenate([f_w1, s_w1], axis=1)          # (32, 128)
    b1cat = np.concatenate([f_b1, s_b1])                  # (128,)
    gamma_tot = np.zeros(D)
    for (p_lo, p_hi, M, gamma) in mats:
        gamma_tot += (p_hi - p_lo + 1) * gamma
    bconst = b1cat.astype(np.float64) + gamma_tot @ W1cat.astype(np.float64)

    W2cat = np.zeros((128, 4), np.float32)
    W2cat[0:64, 0:3] = f_w2
    W2cat[64:128, 3:4] = s_w2
    b2cat = np.concatenate([f_b2, s_b2]).astype(np.float32)
    b2col = np.zeros((128, 1), np.float32)
    for c in range(NSUB):
        b2col[32 * c:32 * c + 4, 0] = b2cat

    # --- packed bf16 constants tensor ---
    cb = np.zeros((128, NB), np.float32)
    # ind: col block 6T+d, row p: node 128T+p in class d
    for dcls, (p_lo, p_hi, M, gamma) in enumerate(mats):
        for node in range(p_lo, p_hi + 1):
            T, p = divmod(node, 128)
            cb[p, IND0 + 6 * T + dcls] = 1.0
    # R fold matrix: quadrant-0 rows 0-5 and quadrant-1 rows 32-37 -> class d
    for dcls in range(6):
        cb[dcls, R0 + dcls] = 1.0
        cb[32 + dcls, R0 + dcls] = 1.0
    # m2 (128, 384): col block k=6*jhi+d : m2[j, 32k+o] = M_d[o, 128*jhi+j]
    for dcls, (p_lo, p_hi, M, gamma) in enumerate(mats):
        Mf = M.astype(np.float32)
        for jhi in range(2):
            k = 6 * jhi + dcls
            cb[:, M20 + 32 * k:M20 + 32 * (k + 1)] = \
                Mf[:, 128 * jhi:128 * (jhi + 1)].T
    # wb2 (65, 128): rows 0-31 W1cat, 32-63 W1cat, 64 bconst
    cb[0:32, WB0:WB0 + 128] = W1cat
    cb[32:64, WB0:WB0 + 128] = W1cat
    cb[64, WB0:WB0 + 128] = bconst.astype(np.float32)
    # w1cat3 (96, 128) and w2 (128, 4)
    cb[0:96, W10:W10 + 128] = np.tile(W1cat, (3, 1))
    cb[:, W20:W20 + 4] = W2cat
    cb = np.ascontiguousarray(cb.astype(ml_dtypes.bfloat16))

    # --- shards ---
    data_flat = data.reshape(N_CELLS, D)

    # replicated scan region (all 4096 parent nodes), bf16
    scan = np.ascontiguousarray(
        data_flat[0:N_GROUPS * 8].reshape(SCAN_TILES, 128, 256).transpose(1, 0, 2)
        .reshape(128, SCAN_FREE).astype(ml_dtypes.bfloat16)
    )

    in_maps = []
    for i in range(CORES):
        base = LEAF0 + CELLS_MAIN * i
        end = min(base + CELLS_CORE, N_CELLS)
        x_lin = np.zeros((CELLS_CORE, D), np.float32)
        x_lin[0:end - base] = data_flat[base:end]
        xT = np.ascontiguousarray(
            x_lin.reshape(NCH, NSUB, SUB, D).transpose(1, 3, 0, 2)
            .reshape(96, XT_FREE).astype(ml_dtypes.bfloat16)
        )
        in_maps.append({
            "cb": cb,
            "scanX": scan,
            "xT": xT,
            "b2col": b2col,
        })

    nc = _get_graph()
    res = bass_utils.run_bass_kernel_spmd(nc, in_maps, core_ids=list(range(CORES)))

    out_flat = np.zeros((N_CELLS, 4), np.float32)
    for i in range(CORES):
        base = LEAF0 + CELLS_MAIN * i
        k = CELLS_MAIN if i < CORES - 1 else CELLS_MAIN + 1
        # planes (12, 9728): row 4c+o holds cells 1536t+512c+cc at free 512t+cc
        planes = res.results[i]["out"].reshape(NSUB, 4, NCH, SUB)  # (c,o,t,cc)
        cells = planes.transpose(2, 0, 3, 1).reshape(CELLS_CORE, 4)  # (t,c,cc),o
        out_flat[base:base + k] = cells[:k]
    return out_flat.reshape(N_NODES, 2, 2, 2, 4)


# revision 21
# speedup vs baseline: 1.0252x; 1.0252x over previous
"""Trainium2 Bass kernel for nn_AdExternal_N3Tree (gnn_message_passing).

Key insight: the reference's sequential 32768-step scan only affects the
output through `acc` (a 32-vector). Each parent's 8-child sibling group is an
independent serial chain that is LINEAR in that parent's original 8x32 block,
and group depth is constant within each of 6 contiguous parent-index classes.
So  acc = sum_d M_d @ s_d + gamma_tot,  where s_d is the sum of flattened
parent blocks over class d (a pure 4MB reduction) and M_d (32x256) / gamma
are tiny matrices computed on the host from conv_w/conv_b/depth_weight alone.

The leaf cells that feed the MLPs (flat cells 32767..262143) are never
written by the scan, so  out[leaf] = MLP(data_flat[leaf] + acc)  and cells
0..32766 are zero.

Device work per core (SPMD on 8 cores, no collectives - each core is fully
independent, which also makes the kernel immune to cross-core launch skew):
  - scan DMA split into 8 slices over the 3 DMA-capable queues (sync HWDGE,
    scalar HWDGE, gpsimd SWDGE) so the class-sum stage starts as slices land
  - stage 1: class sums with two PE quadrants: mixed-class node tiles cycle
    indicator weights in quadrant 0; the 26 pure-class-4 tiles reuse ONE
    resident indicator in quadrant 1 (LDWEIGHTS elision -> feed-bound)
  - tiny chain in bf16: s -> transpose (R-matrix matmuls fold both quadrant
    row groups) -> even/odd acc quadrants -> widened [65,128] bias matmul
    merges them for free -> folded layer-1 bias (128,)
  - MLP over a 29184-cell slice in bf16: x@W1cat (row-tiled) -> GELU+bias on
    ScalarE (the throughput bound, f32 PSUM in) -> @W2cat (col-tiled) ->
    +b2 evict on DVE
  - output written as 12 (c,o)-planes; host interleaves/assembles.
"""
import sys

for _p in ("/opt/trn_rl_repo", "/opt/trn_rl_repo/concourse"):
    if _p not in sys.path:
        sys.path.insert(0, _p)

import numpy as np

N_NODES = 32768
D = 32
N_GROUPS = 4096
N_CELLS = N_NODES * 8           # 262144
LEAF0 = N_NODES - 1             # 32767 first leaf cell
CORES = 8
CELLS_MAIN = 28672              # cells each core contributes (core 7: +1)
CELLS_CORE = 29184              # cells each core processes = 19 * 1536
CH = 1536                       # cells per chunk (3 row-tiled sub-chunks of 512)
NCH = 19
SUB = 512
NSUB = 3
SCAN_TILES = 32                 # replicated scan: 32 node-tiles of 128
SCAN_FREE = SCAN_TILES * 256    # 8192
XT_FREE = NCH * SUB             # 9728

# stage-1 tile classification: tiles fully inside class 4 share one
# indicator (nodes 640..3967 = tiles 5..30); the rest are "mixed"
PURE_LO, PURE_HI = 5, 30
MIXED_TILES = [0, 1, 2, 3, 4, 31]

# constsB (bf16, [128, NB]) column map (ind lives in its own fp8 tensor)
R0 = 0                          # R fold matrix [38, 6]
M20 = R0 + 6                    # 6: m2 [128, 12*32]
WB0 = M20 + 384                 # 390: wb2 [65, 128]
W10 = WB0 + 128                 # 518: w1cat3 [96, 128]
W20 = W10 + 128                 # 646: w2 [128, 4]
NB = W20 + 4                    # 650

# (p_lo, p_hi_inclusive, conv_depth, n_children, extra_j0_step)
CLASSES = [
    (0, 0, 1, 8, True),
    (1, 8, 2, 8, False),
    (9, 72, 3, 8, False),
    (73, 584, 4, 8, False),
    (585, 4094, 5, 8, False),
    (4095, 4095, 5, 7, False),
]


# ---------------------------------------------------------------- host math
def _chain(conv_w_d, conv_b_d, dw_d, n_children):
    W = conv_w_d.astype(np.float64)
    b = conv_b_d.astype(np.float64)
    Wk = [W[:, :, k] for k in range(8)]
    A, beta = {}, {}
    if n_children == 7:
        A7 = np.zeros((8, D, D))
        A7[7] = np.eye(D)
        A[7] = A7
        beta[7] = np.zeros(D)
        cs = range(6, -1, -1)
    else:
        cs = range(7, -1, -1)
    for c in cs:
        Ac = np.zeros((8, D, D))
        bc = b.copy()
        for k in range(0, c + 1):
            Ac[k] += Wk[k]
        for m in range(c + 1, 8):
            for k in range(8):
                Ac[k] += Wk[m] @ A[m][k]
            bc += Wk[m] @ beta[m]
        A[c] = Ac
        beta[c] = bc
    Msum = np.zeros((8, D, D))
    gamma = np.zeros(D)
    for c in (range(8) if n_children == 8 else range(7)):
        Msum += dw_d * A[c]
        gamma += dw_d * beta[c]
    return A, beta, Msum, gamma


def _build_class_mats(conv_w, conv_b, depth_weight):
    out = []
    for (p_lo, p_hi, dep, nch, extra) in CLASSES:
        A, beta, Msum, gamma = _chain(
            conv_w[dep], conv_b[dep], float(depth_weight[dep]), nch
        )
        if extra:
            W0 = conv_w[0].astype(np.float64)
            b0 = conv_b[0].astype(np.float64)
            W0k = [W0[:, :, k] for k in range(8)]
            Ae = np.zeros((8, D, D))
            be = b0.copy()
            for m in range(8):
                for k in range(8):
                    Ae[k] += W0k[m] @ A[m][k]
                be += W0k[m] @ beta[m]
            Msum = Msum + float(depth_weight[0]) * Ae
            gamma = gamma + float(depth_weight[0]) * be
        M = np.concatenate([Msum[k] for k in range(8)], axis=1)  # (D, 8D)
        out.append((p_lo, p_hi, M, gamma))
    return out


# ---------------------------------------------------------------- device graph
_GRAPH = None


def _build_graph():
    import concourse.bacc as bacc
    import concourse.mybir as mybir
    from concourse import tile
    from concourse.tile_rust import add_dep_helper

    F32 = mybir.dt.float32
    BF16 = mybir.dt.bfloat16
    nc = bacc.Bacc("TRN2", target_bir_lowering=False, debug=False, num_devices=CORES)

    cb_d = nc.declare_dram_parameter("cb", [128, NB], BF16, isOutput=False)
    ind_d = nc.declare_dram_parameter("ind8", [128, 192], BF16, isOutput=False)
    scan_d = nc.declare_dram_parameter("scanX", [128, SCAN_FREE], BF16, isOutput=False)
    xT_d = nc.declare_dram_parameter("xT", [96, XT_FREE], BF16, isOutput=False)
    b2_d = nc.declare_dram_parameter("b2col", [128, 1], F32, isOutput=False)
    out_d = nc.declare_dram_parameter("out", [12, XT_FREE], F32, isOutput=True)

    Gelu = mybir.ActivationFunctionType.Gelu

    with tile.TileContext(nc) as tc:
        with (
            tc.tile_pool(name="const", bufs=1) as cpool,
            tc.tile_pool(name="data", bufs=1) as dpool,
            tc.tile_pool(name="gp", bufs=3) as gpool,
        ):
            # ACT warm-up (gelu table load) + PE warm-up source, no DMA deps
            warm_sb = cpool.tile([1, 8], F32)
            nc.gpsimd.memset(warm_sb[:], 0.0)
            nc.scalar.activation(warm_sb[:], warm_sb[:], Gelu)
            warmd_sb = cpool.tile([32, 128], BF16)
            nc.gpsimd.memset(warmd_sb[:], 0.001)

            cb_sb = cpool.tile([128, NB], BF16)
            ind_sb = cpool.tile([128, 192], BF16)
            b2_sb = cpool.tile([128, 1], F32)
            acc1 = cpool.tile([65, 1], BF16)
            nc.gpsimd.memset(acc1[64:65, :], 1.0)
            bias_sb = cpool.tile([128, 1], F32)
            s_sb = cpool.tile([38, 256], BF16)
            sT_sb = cpool.tile([128, 12], BF16)

            scan_sb = dpool.tile([128, SCAN_FREE], BF16)
            xT_sb = dpool.tile([96, XT_FREE], BF16)
            stage_sb = dpool.tile([128, XT_FREE], F32)

            # ---- DMA enqueues ----
            # ind + consts first on scalar (needed by stage 1); fp8 scan
            # slices interleave across sync/gpsimd in consumption order; xT
            # quarters wait for the whole scan (keeps the bias critical path
            # at full DMA bandwidth); b2col last (needed at ~first add)
            nc.scalar.dma_start(ind_sb[:], ind_d.ap())
            nc.scalar.dma_start(cb_sb[:], cb_d.ap())
            SLICE_Q = [nc.sync, nc.gpsimd, nc.sync, nc.gpsimd,
                       nc.sync, nc.gpsimd, nc.sync, nc.sync]
            scan_dmas = []
            for k, eng in enumerate(SLICE_Q):
                scan_dmas.append(eng.dma_start(
                    scan_sb[:, 1024 * k:1024 * (k + 1)],
                    scan_d.ap()[:, 1024 * k:1024 * (k + 1)],
                ))
            # xT: a small early slice (chunks 0-2) rides gpsimd with no dep
            # so z0-z2 can prefill during the chain; the remaining three
            # pieces wait for the scan (bias critical path owns the HBM)
            nc.gpsimd.dma_start(xT_sb[:, 0:1536], xT_d.ap()[:, 0:1536])
            XT_PIECES = [(1536, 4096, nc.sync), (4096, 6656, nc.gpsimd),
                         (6656, 9728, nc.sync)]
            for lo, hi, eng in XT_PIECES:
                xi = eng.dma_start(xT_sb[:, lo:hi], xT_d.ap()[:, lo:hi])
                for sd in scan_dmas:
                    add_dep_helper(xi.ins, sd.ins, sync=True,
                                   reason="serialize xT behind scan")
            nc.scalar.dma_start(b2_sb[:], b2_d.ap())

            with tc.tile_pool(name="psZ", bufs=2, space="PSUM") as zp:
                with tc.tile_pool(name="psC", bufs=1, space="PSUM") as pchain:
                    # chain PSUM lives in ONE bank: cols 0:256 stage-1 class
                    # sums (+ warm-up junk), 256:268 sT, 268:269 acc E/O,
                    # 269:270 bias
                    ps_part = pchain.tile([128, 272], F32)

                    # PE pre-warm: open the HAM clock gate before stage 1
                    for _ in range(16):
                        nc.tensor.matmul(
                            ps_part[:, 0:128], warmd_sb[:], warmd_sb[:],
                            start=True, stop=True,
                        )

                    # stage 1: class sums over the replicated scan region.
                    # mixed tiles cycle indicators in quadrant 0 (rows 0-5);
                    # pure class-4 tiles share tile-5's indicator resident in
                    # quadrant 1 (rows 32-37) -> no LDWEIGHTS between them
                    for T in range(SCAN_TILES):
                        pure = PURE_LO <= T <= PURE_HI
                        ind_T = 6 * PURE_LO if pure else 6 * T
                        nc.tensor.matmul(
                            ps_part[32:38, 0:256] if pure else ps_part[0:6, 0:256],
                            ind_sb[:, ind_T:ind_T + 6],
                            scan_sb[:, 256 * T:256 * (T + 1)],
                            start=(T == PURE_LO if pure else T == 0),
                            stop=(T == PURE_HI if pure else T == SCAN_TILES - 1),
                            tile_position=(0, 32) if pure else (0, 0),
                        )

                    # s (38,256) -> sT (128,12) via R-matmuls that also fold
                    # the two quadrant row groups (R[d,d]=R[32+d,d]=1)
                    nc.vector.tensor_copy(s_sb[:], ps_part[0:38, 0:256])
                    for jhi in range(2):
                        nc.tensor.matmul(
                            ps_part[:, 256 + 6 * jhi:256 + 6 * jhi + 6],
                            s_sb[:, 128 * jhi:128 * (jhi + 1)],
                            cb_sb[0:38, R0:R0 + 6],
                            start=True, stop=True,
                        )
                    nc.vector.tensor_copy(sT_sb[:], ps_part[:, 256:268])

                    # acc = sum_k M2_k @ sT[:, k], even k in quadrant 0
                    # (rows 0:32), odd k in quadrant 1 (rows 32:64)
                    for k in range(12):
                        odd = k % 2
                        nc.tensor.matmul(
                            ps_part[32 * odd:32 * odd + 32, 268:269],
                            cb_sb[:, M20 + 32 * k:M20 + 32 * (k + 1)],
                            sT_sb[:, k:k + 1],
                            start=(k < 2), stop=(k >= 10),
                            tile_position=(0, 32 * odd),
                        )
                    nc.vector.tensor_copy(acc1[0:64, :], ps_part[0:64, 268:269])

                    # bias1_eff = W1cat.T@(accE+accO) + (b1cat + gamma@W1cat)
                    # via the widened [65,128] wb2 (rows 0-31 W1, 32-63 W1,
                    # 64 bconst)
                    nc.tensor.matmul(
                        ps_part[:, 269:270], cb_sb[0:65, WB0:WB0 + 128],
                        acc1[:], start=True, stop=True,
                    )
                    nc.vector.tensor_copy(bias_sb[:], ps_part[:, 269:270])

                with tc.tile_pool(name="psO", bufs=2, space="PSUM") as op:
                    for t in range(NCH):
                        z = zp.tile([128, CH], F32)
                        for a in range(NSUB):
                            nc.tensor.matmul(
                                z[:, SUB * a:SUB * (a + 1)],
                                cb_sb[32 * a:32 * (a + 1), W10:W10 + 128],
                                xT_sb[32 * a:32 * (a + 1), SUB * t:SUB * (t + 1)],
                                start=True,
                                stop=True,
                                tile_position=(32 * a, 0),
                            )
                        g = gpool.tile([128, CH], BF16)
                        nc.scalar.activation(g[:], z[:], Gelu, bias=bias_sb[:])
                        o_ps = op.tile([128, SUB], F32)
                        for c in range(NSUB):
                            nc.tensor.matmul(
                                o_ps[32 * c:32 * c + 4, :],
                                cb_sb[:, W20:W20 + 4],
                                g[:, SUB * c:SUB * (c + 1)],
                                start=True,
                                stop=True,
                                tile_position=(0, 32 * c),
                            )
                        nc.vector.tensor_scalar_add(
                            stage_sb[:, SUB * t:SUB * (t + 1)], o_ps[:], b2_sb[:]
                        )
                        # batched output DMA on the idle gpsimd queue; the
                        # final (small) batch fans one strip to each of the
                        # 3 queues so their completion latencies overlap
                        if t in (4, 9, 13, 16, 17, NCH - 1):
                            lo = {4: 0, 9: 2560, 13: 5120, 16: 7168,
                                  17: 8704, NCH - 1: 9216}[t]
                            hi = SUB * (t + 1)
                            engs = ([nc.gpsimd] * 3 if t != NCH - 1
                                    else [nc.sync, nc.gpsimd, nc.scalar])
                            for c in range(NSUB):
                                engs[c].dma_start(
                                    out_d.ap()[4 * c:4 * c + 4, lo:hi],
                                    stage_sb[32 * c:32 * c + 4, lo:hi],
                                )

    nc.compile()
    return nc


def _get_graph():
    global _GRAPH
    if _GRAPH is None:
        _GRAPH = _build_graph()
    return _GRAPH


# ---------------------------------------------------------------- kernel
def kernel(**inputs):
    import ml_dtypes
    from concourse import bass_utils

    data = np.asarray(inputs["data"], np.float32)
    conv_w = np.asarray(inputs["conv_w"], np.float32)
    conv_b = np.asarray(inputs["conv_b"], np.float32)
    dw = np.asarray(inputs["depth_weight"], np.float32)
    f_w1 = np.asarray(inputs["f_w1"], np.float32)
    f_b1 = np.asarray(inputs["f_b1"], np.float32)
    f_w2 = np.asarray(inputs["f_w2"], np.float32)
    f_b2 = np.asarray(inputs["f_b2"], np.float32)
    s_w1 = np.asarray(inputs["s_w1"], np.float32)
    s_b1 = np.asarray(inputs["s_b1"], np.float32)
    s_w2 = np.asarray(inputs["s_w2"], np.float32)
    s_b2 = np.asarray(inputs["s_b2"], np.float32)

    # --- weight-derived host constants (no data-sized work here) ---
    mats = _build_class_mats(conv_w, conv_b, dw)

    W1cat = np.concatenate([f_w1, s_w1], axis=1)          # (32, 128)
    b1cat = np.concatenate([f_b1, s_b1])                  # (128,)
    gamma_tot = np.zeros(D)
    for (p_lo, p_hi, M, gamma) in mats:
        gamma_tot += (p_hi - p_lo + 1) * gamma
    bconst = b1cat.astype(np.float64) + gamma_tot @ W1cat.astype(np.float64)

    W2cat = np.zeros((128, 4), np.float32)
    W2cat[0:64, 0:3] = f_w2
    W2cat[64:128, 3:4] = s_w2
    b2cat = np.concatenate([f_b2, s_b2]).astype(np.float32)
    b2col = np.zeros((128, 1), np.float32)
    for c in range(NSUB):
        b2col[32 * c:32 * c + 4, 0] = b2cat

    # --- packed constants ---
    # ind (fp8): col block 6T+d, row p: node 128T+p in class d
    ind8 = np.zeros((128, 192), np.float32)
    for dcls, (p_lo, p_hi, M, gamma) in enumerate(mats):
        for node in range(p_lo, p_hi + 1):
            T, p = divmod(node, 128)
            ind8[p, 6 * T + dcls] = 1.0
    ind8 = np.ascontiguousarray(ind8.astype(ml_dtypes.bfloat16))

    cb = np.zeros((128, NB), np.float32)
    # R fold matrix: quadrant-0 rows 0-5 and quadrant-1 rows 32-37 -> class d
    for dcls in range(6):
        cb[dcls, R0 + dcls] = 1.0
        cb[32 + dcls, R0 + dcls] = 1.0
    # m2 (128, 384): col block k=6*jhi+d : m2[j, 32k+o] = M_d[o, 128*jhi+j]
    for dcls, (p_lo, p_hi, M, gamma) in enumerate(mats):
        Mf = M.astype(np.float32)
        for jhi in range(2):
            k = 6 * jhi + dcls
            cb[:, M20 + 32 * k:M20 + 32 * (k + 1)] = \
                Mf[:, 128 * jhi:128 * (jhi + 1)].T
    # wb2 (65, 128): rows 0-31 W1cat, 32-63 W1cat, 64 bconst
    cb[0:32, WB0:WB0 + 128] = W1cat
    cb[32:64, WB0:WB0 + 128] = W1cat
    cb[64, WB0:WB0 + 128] = bconst.astype(np.float32)
    # w1cat3 (96, 128) and w2 (128, 4)
    cb[0:96, W10:W10 + 128] = np.tile(W1cat, (3, 1))
    cb[:, W20:W20 + 4] = W2cat
    cb = np.ascontiguousarray(cb.astype(ml_dtypes.bfloat16))

    # --- shards ---
    data_flat = data.reshape(N_CELLS, D)

    # replicated scan region (all 4096 parent nodes), bf16
    scan = np.ascontiguousarray(
        data_flat[0:N_GROUPS * 8].reshape(SCAN_TILES, 128, 256).transpose(1, 0, 2)
        .reshape(128, SCAN_FREE).astype(ml_dtypes.bfloat16)
    )

    in_maps = []
    for i in range(CORES):
        base = LEAF0 + CELLS_MAIN * i
        end = min(base + CELLS_CORE, N_CELLS)
        x_lin = np.zeros((CELLS_CORE, D), np.float32)
        x_lin[0:end - base] = data_flat[base:end]
        xT = np.ascontiguousarray(
            x_lin.reshape(NCH, NSUB, SUB, D).transpose(1, 3, 0, 2)
            .reshape(96, XT_FREE).astype(ml_dtypes.bfloat16)
        )
        in_maps.append({
            "cb": cb,
            "ind8": ind8,
            "scanX": scan,
            "xT": xT,
            "b2col": b2col,
        })

    nc = _get_graph()
    res = bass_utils.run_bass_kernel_spmd(nc, in_maps, core_ids=list(range(CORES)))

    out_flat = np.zeros((N_CELLS, 4), np.float32)
    for i in range(CORES):
        base = LEAF0 + CELLS_MAIN * i
        k = CELLS_MAIN if i < CORES - 1 else CELLS_MAIN + 1
        # planes (12, 9728): row 4c+o holds cells 1536t+512c+cc at free 512t+cc
        planes = res.results[i]["out"].reshape(NSUB, 4, NCH, SUB)  # (c,o,t,cc)
        cells = planes.transpose(2, 0, 3, 1).reshape(CELLS_CORE, 4)  # (t,c,cc),o
        out_flat[base:base + k] = cells[:k]
    return out_flat.reshape(N_NODES, 2, 2, 2, 4)


# revision 25
# speedup vs baseline: 1.0333x; 1.0079x over previous
"""Trainium2 Bass kernel for nn_AdExternal_N3Tree (gnn_message_passing).

Key insight: the reference's sequential 32768-step scan only affects the
output through `acc` (a 32-vector). Each parent's 8-child sibling group is an
independent serial chain that is LINEAR in that parent's original 8x32 block,
and group depth is constant within each of 6 contiguous parent-index classes.
So  acc = sum_d M_d @ s_d + gamma_tot,  where s_d is the sum of flattened
parent blocks over class d (a pure 4MB reduction) and M_d (32x256) / gamma
are tiny matrices computed on the host from conv_w/conv_b/depth_weight alone.

The leaf cells that feed the MLPs (flat cells 32767..262143) are never
written by the scan, so  out[leaf] = MLP(data_flat[leaf] + acc)  and cells
0..32766 are zero.

Device work per core (SPMD on 8 cores, no collectives - each core is fully
independent, which also makes the kernel immune to cross-core launch skew):
  - scan DMA split into 8 slices over the 3 DMA-capable queues (sync HWDGE,
    scalar HWDGE, gpsimd SWDGE) so the class-sum stage starts as slices land
  - stage 1: class sums with two PE quadrants: mixed-class node tiles cycle
    indicator weights in quadrant 0; the 26 pure-class-4 tiles reuse ONE
    resident indicator in quadrant 1 (LDWEIGHTS elision -> feed-bound)
  - tiny chain in bf16: s -> transpose (R-matrix matmuls fold both quadrant
    row groups) -> even/odd acc quadrants -> widened [65,128] bias matmul
    merges them for free -> folded layer-1 bias (128,)
  - MLP over a 29184-cell slice in bf16: x@W1cat (row-tiled) -> GELU+bias on
    ScalarE (the throughput bound, f32 PSUM in) -> @W2cat (col-tiled) ->
    +b2 evict on DVE
  - output written as 12 (c,o)-planes; host interleaves/assembles.
"""
import sys

for _p in ("/opt/trn_rl_repo", "/opt/trn_rl_repo/concourse"):
    if _p not in sys.path:
        sys.path.insert(0, _p)

import numpy as np

N_NODES = 32768
D = 32
N_GROUPS = 4096
N_CELLS = N_NODES * 8           # 262144
LEAF0 = N_NODES - 1             # 32767 first leaf cell
CORES = 8
CELLS_MAIN = 28672              # cells each core contributes (core 7: +1)
CELLS_CORE = 29184              # cells each core processes = 19 * 1536
CH = 1536                       # cells per chunk (3 row-tiled sub-chunks of 512)
NCH = 19
SUB = 512
NSUB = 3
SCAN_TILES = 32                 # replicated scan: 32 node-tiles of 128
SCAN_FREE = SCAN_TILES * 256    # 8192
XT_FREE = NCH * SUB             # 9728

# stage-1 tile classification: tiles fully inside class 4 share one
# indicator (nodes 640..3967 = tiles 5..30); the rest are "mixed"
PURE_LO, PURE_HI = 5, 30
MIXED_TILES = [0, 1, 2, 3, 4, 31]

# constsB (bf16, [128, NB]) column map (ind lives in its own fp8 tensor)
R0 = 0                          # R fold matrix [38, 6]
M20 = R0 + 6                    # 6: m2 [128, 12*32]
WB0 = M20 + 384                 # 390: wb2 [65, 128]
W10 = WB0 + 128                 # 518: w1cat3 [96, 128]
W20 = W10 + 128                 # 646: w2 [128, 4]
NB = W20 + 4                    # 650

# (p_lo, p_hi_inclusive, conv_depth, n_children, extra_j0_step)
CLASSES = [
    (0, 0, 1, 8, True),
    (1, 8, 2, 8, False),
    (9, 72, 3, 8, False),
    (73, 584, 4, 8, False),
    (585, 4094, 5, 8, False),
    (4095, 4095, 5, 7, False),
]


# ---------------------------------------------------------------- host math
def _chain(conv_w_d, conv_b_d, dw_d, n_children):
    W = conv_w_d.astype(np.float64)
    b = conv_b_d.astype(np.float64)
    Wk = [W[:, :, k] for k in range(8)]
    A, beta = {}, {}
    if n_children == 7:
        A7 = np.zeros((8, D, D))
        A7[7] = np.eye(D)
        A[7] = A7
        beta[7] = np.zeros(D)
        cs = range(6, -1, -1)
    else:
        cs = range(7, -1, -1)
    for c in cs:
        Ac = np.zeros((8, D, D))
        bc = b.copy()
        for k in range(0, c + 1):
            Ac[k] += Wk[k]
        for m in range(c + 1, 8):
            for k in range(8):
                Ac[k] += Wk[m] @ A[m][k]
            bc += Wk[m] @ beta[m]
        A[c] = Ac
        beta[c] = bc
    Msum = np.zeros((8, D, D))
    gamma = np.zeros(D)
    for c in (range(8) if n_children == 8 else range(7)):
        Msum += dw_d * A[c]
        gamma += dw_d * beta[c]
    return A, beta, Msum, gamma


def _build_class_mats(conv_w, conv_b, depth_weight):
    out = []
    for (p_lo, p_hi, dep, nch, extra) in CLASSES:
        A, beta, Msum, gamma = _chain(
            conv_w[dep], conv_b[dep], float(depth_weight[dep]), nch
        )
        if extra:
            W0 = conv_w[0].astype(np.float64)
            b0 = conv_b[0].astype(np.float64)
            W0k = [W0[:, :, k] for k in range(8)]
            Ae = np.zeros((8, D, D))
            be = b0.copy()
            for m in range(8):
                for k in range(8):
                    Ae[k] += W0k[m] @ A[m][k]
                be += W0k[m] @ beta[m]
            Msum = Msum + float(depth_weight[0]) * Ae
            gamma = gamma + float(depth_weight[0]) * be
        M = np.concatenate([Msum[k] for k in range(8)], axis=1)  # (D, 8D)
        out.append((p_lo, p_hi, M, gamma))
    return out


# ---------------------------------------------------------------- device graph
_GRAPH = None


def _build_graph():
    import concourse.bacc as bacc
    import concourse.mybir as mybir
    from concourse import tile
    from concourse.tile_rust import add_dep_helper

    F32 = mybir.dt.float32
    BF16 = mybir.dt.bfloat16
    nc = bacc.Bacc("TRN2", target_bir_lowering=False, debug=False, num_devices=CORES)

    cb_d = nc.declare_dram_parameter("cb", [128, NB], BF16, isOutput=False)
    ind_d = nc.declare_dram_parameter("ind8", [128, 192], BF16, isOutput=False)
    # scan/xT split into separate DRAM tensors so each transfer reads
    # CONTIGUOUS DRAM (a column-slice of one big tensor is 16KB-strided
    # 2KB chunks, which halves effective HBM bandwidth)
    scan_ds = [
        nc.declare_dram_parameter(f"scan{k}", [128, 1024], BF16, isOutput=False)
        for k in range(8)
    ]
    XT_BOUNDS = [0, 1536, 4096, 6656, XT_FREE]
    xT_ds = [
        nc.declare_dram_parameter(
            f"xT{q}", [96, XT_BOUNDS[q + 1] - XT_BOUNDS[q]], BF16, isOutput=False
        )
        for q in range(4)
    ]
    b2_d = nc.declare_dram_parameter("b2col", [128, 1], F32, isOutput=False)
    out_d = nc.declare_dram_parameter("out", [12, XT_FREE], F32, isOutput=True)

    Gelu = mybir.ActivationFunctionType.Gelu

    with tile.TileContext(nc) as tc:
        with (
            tc.tile_pool(name="const", bufs=1) as cpool,
            tc.tile_pool(name="data", bufs=1) as dpool,
            tc.tile_pool(name="gp", bufs=3) as gpool,
        ):
            # ACT warm-up (gelu table load) + PE warm-up source, no DMA deps
            warm_sb = cpool.tile([1, 8], F32)
            nc.gpsimd.memset(warm_sb[:], 0.0)
            nc.scalar.activation(warm_sb[:], warm_sb[:], Gelu)
            warmd_sb = cpool.tile([32, 128], BF16)
            nc.gpsimd.memset(warmd_sb[:], 0.001)

            cb_sb = cpool.tile([128, NB], BF16)
            ind_sb = cpool.tile([128, 192], BF16)
            b2_sb = cpool.tile([128, 1], F32)
            acc1 = cpool.tile([65, 1], BF16)
            nc.gpsimd.memset(acc1[64:65, :], 1.0)
            bias_sb = cpool.tile([128, 1], F32)
            s_sb = cpool.tile([38, 256], BF16)
            sT_sb = cpool.tile([128, 12], BF16)

            scan_sb = dpool.tile([128, SCAN_FREE], BF16)
            xT_sb = dpool.tile([96, XT_FREE], BF16)
            stage_sb = dpool.tile([128, XT_FREE], F32)

            # ---- DMA enqueues ----
            # ind + consts first on scalar (needed by stage 1); fp8 scan
            # slices interleave across sync/gpsimd in consumption order; xT
            # quarters wait for the whole scan (keeps the bias critical path
            # at full DMA bandwidth); b2col last (needed at ~first add)
            nc.scalar.dma_start(ind_sb[:], ind_d.ap())
            nc.scalar.dma_start(cb_sb[:], cb_d.ap())
            SLICE_Q = [nc.sync, nc.gpsimd, nc.sync, nc.gpsimd,
                       nc.sync, nc.gpsimd, nc.sync, nc.gpsimd]
            scan_dmas = []
            for k, eng in enumerate(SLICE_Q):
                scan_dmas.append(eng.dma_start(
                    scan_sb[:, 1024 * k:1024 * (k + 1)],
                    scan_ds[k].ap(),
                ))
            # xT: a small early piece (chunks 0-2) rides gpsimd with no dep
            # so z0-z2 can prefill during the chain; the remaining three
            # pieces wait for the scan (bias critical path owns the HBM)
            XT_PIECE_Q = [nc.gpsimd, nc.sync, nc.gpsimd, nc.sync]
            for q, eng in enumerate(XT_PIECE_Q):
                lo, hi = XT_BOUNDS[q], XT_BOUNDS[q + 1]
                xi = eng.dma_start(xT_sb[:, lo:hi], xT_ds[q].ap())
                if q > 0:
                    for sd in scan_dmas:
                        add_dep_helper(xi.ins, sd.ins, sync=True,
                                       reason="serialize xT behind scan")
            nc.scalar.dma_start(b2_sb[:], b2_d.ap())

            with tc.tile_pool(name="psZ", bufs=2, space="PSUM") as zp:
                with tc.tile_pool(name="psC", bufs=1, space="PSUM") as pchain:
                    # chain PSUM lives in ONE bank: cols 0:256 stage-1 class
                    # sums (+ warm-up junk), 256:268 sT, 268:269 acc E/O,
                    # 269:270 bias
                    ps_part = pchain.tile([128, 272], F32)

                    # PE pre-warm: open the HAM clock gate before stage 1
                    for _ in range(16):
                        nc.tensor.matmul(
                            ps_part[:, 0:128], warmd_sb[:], warmd_sb[:],
                            start=True, stop=True,
                        )

                    # stage 1: class sums over the replicated scan region.
                    # mixed tiles cycle indicators in quadrant 0 (rows 0-5);
                    # pure class-4 tiles share tile-5's indicator resident in
                    # quadrant 1 (rows 32-37) -> no LDWEIGHTS between them
                    for T in range(SCAN_TILES):
                        pure = PURE_LO <= T <= PURE_HI
                        ind_T = 6 * PURE_LO if pure else 6 * T
                        nc.tensor.matmul(
                            ps_part[32:38, 0:256] if pure else ps_part[0:6, 0:256],
                            ind_sb[:, ind_T:ind_T + 6],
                            scan_sb[:, 256 * T:256 * (T + 1)],
                            start=(T == PURE_LO if pure else T == 0),
                            stop=(T == PURE_HI if pure else T == SCAN_TILES - 1),
                            tile_position=(0, 32) if pure else (0, 0),
                        )

                    # s (38,256) -> sT (128,12) via R-matmuls that also fold
                    # the two quadrant row groups (R[d,d]=R[32+d,d]=1)
                    nc.vector.tensor_copy(s_sb[:], ps_part[0:38, 0:256])
                    for jhi in range(2):
                        nc.tensor.matmul(
                            ps_part[:, 256 + 6 * jhi:256 + 6 * jhi + 6],
                            s_sb[:, 128 * jhi:128 * (jhi + 1)],
                            cb_sb[0:38, R0:R0 + 6],
                            start=True, stop=True,
                        )
                    nc.vector.tensor_copy(sT_sb[:], ps_part[:, 256:268])

                    # acc = sum_k M2_k @ sT[:, k], even k in quadrant 0
                    # (rows 0:32), odd k in quadrant 1 (rows 32:64)
                    for k in range(12):
                        odd = k % 2
                        nc.tensor.matmul(
                            ps_part[32 * odd:32 * odd + 32, 268:269],
                            cb_sb[:, M20 + 32 * k:M20 + 32 * (k + 1)],
                            sT_sb[:, k:k + 1],
                            start=(k < 2), stop=(k >= 10),
                            tile_position=(0, 32 * odd),
                        )
                    nc.vector.tensor_copy(acc1[0:64, :], ps_part[0:64, 268:269])

                    # bias1_eff = W1cat.T@(accE+accO) + (b1cat + gamma@W1cat)
                    # via the widened [65,128] wb2 (rows 0-31 W1, 32-63 W1,
                    # 64 bconst)
                    nc.tensor.matmul(
                        ps_part[:, 269:270], cb_sb[0:65, WB0:WB0 + 128],
                        acc1[:], start=True, stop=True,
                    )
                    nc.vector.tensor_copy(bias_sb[:], ps_part[:, 269:270])

                with tc.tile_pool(name="psO", bufs=2, space="PSUM") as op:
                    for t in range(NCH):
                        z = zp.tile([128, CH], F32)
                        for a in range(NSUB):
                            nc.tensor.matmul(
                                z[:, SUB * a:SUB * (a + 1)],
                                cb_sb[32 * a:32 * (a + 1), W10:W10 + 128],
                                xT_sb[32 * a:32 * (a + 1), SUB * t:SUB * (t + 1)],
                                start=True,
                                stop=True,
                                tile_position=(32 * a, 0),
                            )
                        g = gpool.tile([128, CH], BF16)
                        nc.scalar.activation(g[:], z[:], Gelu, bias=bias_sb[:])
                        o_ps = op.tile([128, SUB], F32)
                        for c in range(NSUB):
                            nc.tensor.matmul(
                                o_ps[32 * c:32 * c + 4, :],
                                cb_sb[:, W20:W20 + 4],
                                g[:, SUB * c:SUB * (c + 1)],
                                start=True,
                                stop=True,
                                tile_position=(0, 32 * c),
                            )
                        nc.vector.tensor_scalar_add(
                            stage_sb[:, SUB * t:SUB * (t + 1)], o_ps[:], b2_sb[:]
                        )
                        # batched output DMA on the idle gpsimd queue; the
                        # final (small) batch fans one strip to each of the
                        # 3 queues so their completion latencies overlap
                        if t in (4, 9, 13, 16, 17, NCH - 1):
                            lo = {4: 0, 9: 2560, 13: 5120, 16: 7168,
                                  17: 8704, NCH - 1: 9216}[t]
                            hi = SUB * (t + 1)
                            engs = ([nc.gpsimd] * 3 if t != NCH - 1
                                    else [nc.sync, nc.gpsimd, nc.scalar])
                            for c in range(NSUB):
                                engs[c].dma_start(
                                    out_d.ap()[4 * c:4 * c + 4, lo:hi],
                                    stage_sb[32 * c:32 * c + 4, lo:hi],
                                )

    nc.compile()
    return nc


def _get_graph():
    global _GRAPH
    if _GRAPH is None:
        _GRAPH = _build_graph()
    return _GRAPH


# ---------------------------------------------------------------- kernel
def kernel(**inputs):
    import ml_dtypes
    from concourse import bass_utils

    data = np.asarray(inputs["data"], np.float32)
    conv_w = np.asarray(inputs["conv_w"], np.float32)
    conv_b = np.asarray(inputs["conv_b"], np.float32)
    dw = np.asarray(inputs["depth_weight"], np.float32)
    f_w1 = np.asarray(inputs["f_w1"], np.float32)
    f_b1 = np.asarray(inputs["f_b1"], np.float32)
    f_w2 = np.asarray(inputs["f_w2"], np.float32)
    f_b2 = np.asarray(inputs["f_b2"], np.float32)
    s_w1 = np.asarray(inputs["s_w1"], np.float32)
    s_b1 = np.asarray(inputs["s_b1"], np.float32)
    s_w2 = np.asarray(inputs["s_w2"], np.float32)
    s_b2 = np.asarray(inputs["s_b2"], np.float32)

    # --- weight-derived host constants (no data-sized work here) ---
    mats = _build_class_mats(conv_w, conv_b, dw)

    W1cat = np.concatenate([f_w1, s_w1], axis=1)          # (32, 128)
    b1cat = np.concatenate([f_b1, s_b1])                  # (128,)
    gamma_tot = np.zeros(D)
    for (p_lo, p_hi, M, gamma) in mats:
        gamma_tot += (p_hi - p_lo + 1) * gamma
    bconst = b1cat.astype(np.float64) + gamma_tot @ W1cat.astype(np.float64)

    W2cat = np.zeros((128, 4), np.float32)
    W2cat[0:64, 0:3] = f_w2
    W2cat[64:128, 3:4] = s_w2
    b2cat = np.concatenate([f_b2, s_b2]).astype(np.float32)
    b2col = np.zeros((128, 1), np.float32)
    for c in range(NSUB):
        b2col[32 * c:32 * c + 4, 0] = b2cat

    # --- packed constants ---
    # ind (fp8): col block 6T+d, row p: node 128T+p in class d
    ind8 = np.zeros((128, 192), np.float32)
    for dcls, (p_lo, p_hi, M, gamma) in enumerate(mats):
        for node in range(p_lo, p_hi + 1):
            T, p = divmod(node, 128)
            ind8[p, 6 * T + dcls] = 1.0
    ind8 = np.ascontiguousarray(ind8.astype(ml_dtypes.bfloat16))

    cb = np.zeros((128, NB), np.float32)
    # R fold matrix: quadrant-0 rows 0-5 and quadrant-1 rows 32-37 -> class d
    for dcls in range(6):
        cb[dcls, R0 + dcls] = 1.0
        cb[32 + dcls, R0 + dcls] = 1.0
    # m2 (128, 384): col block k=6*jhi+d : m2[j, 32k+o] = M_d[o, 128*jhi+j]
    for dcls, (p_lo, p_hi, M, gamma) in enumerate(mats):
        Mf = M.astype(np.float32)
        for jhi in range(2):
            k = 6 * jhi + dcls
            cb[:, M20 + 32 * k:M20 + 32 * (k + 1)] = \
                Mf[:, 128 * jhi:128 * (jhi + 1)].T
    # wb2 (65, 128): rows 0-31 W1cat, 32-63 W1cat, 64 bconst
    cb[0:32, WB0:WB0 + 128] = W1cat
    cb[32:64, WB0:WB0 + 128] = W1cat
    cb[64, WB0:WB0 + 128] = bconst.astype(np.float32)
    # w1cat3 (96, 128) and w2 (128, 4)
    cb[0:96, W10:W10 + 128] = np.tile(W1cat, (3, 1))
    cb[:, W20:W20 + 4] = W2cat
    cb = np.ascontiguousarray(cb.astype(ml_dtypes.bfloat16))

    # --- shards ---
    data_flat = data.reshape(N_CELLS, D)

    # replicated scan region (all 4096 parent nodes), bf16, one contiguous
    # array per DMA slice
    scan = (
        data_flat[0:N_GROUPS * 8].reshape(SCAN_TILES, 128, 256).transpose(1, 0, 2)
        .reshape(128, SCAN_FREE).astype(ml_dtypes.bfloat16)
    )
    scan_slices = [
        np.ascontiguousarray(scan[:, 1024 * k:1024 * (k + 1)]) for k in range(8)
    ]
    XT_BOUNDS = [0, 1536, 4096, 6656, XT_FREE]

    in_maps = []
    for i in range(CORES):
        base = LEAF0 + CELLS_MAIN * i
        end = min(base + CELLS_CORE, N_CELLS)
        x_lin = np.zeros((CELLS_CORE, D), np.float32)
        x_lin[0:end - base] = data_flat[base:end]
        xT = (
            x_lin.reshape(NCH, NSUB, SUB, D).transpose(1, 3, 0, 2)
            .reshape(96, XT_FREE).astype(ml_dtypes.bfloat16)
        )
        im = {"cb": cb, "ind8": ind8, "b2col": b2col}
        for k in range(8):
            im[f"scan{k}"] = scan_slices[k]
        for q in range(4):
            im[f"xT{q}"] = np.ascontiguousarray(
                xT[:, XT_BOUNDS[q]:XT_BOUNDS[q + 1]]
            )
        in_maps.append(im)

    nc = _get_graph()
    res = bass_utils.run_bass_kernel_spmd(nc, in_maps, core_ids=list(range(CORES)))

    out_flat = np.zeros((N_CELLS, 4), np.float32)
    for i in range(CORES):
        base = LEAF0 + CELLS_MAIN * i
        k = CELLS_MAIN if i < CORES - 1 else CELLS_MAIN + 1
        # planes (12, 9728): row 4c+o holds cells 1536t+512c+cc at free 512t+cc
        planes = res.results[i]["out"].reshape(NSUB, 4, NCH, SUB)  # (c,o,t,cc)
        cells = planes.transpose(2, 0, 3, 1).reshape(CELLS_CORE, 4)  # (t,c,cc),o
        out_flat[base:base + k] = cells[:k]
    return out_flat.reshape(N_NODES, 2, 2, 2, 4)
